# revision 21
# baseline (speedup 1.0000x reference)
"""AFT-Local sparse attention kernel for Trainium2, SPMD over 8 NeuronCores.

Problem (B=4, L=1024, E=256, S=32):
    Q = q @ Wq.T + bq ; K = q @ Wk.T + bk ; V = q @ Wv.T + bv
    For each (b, i, e):  per-channel softmax over the 65-wide window
        logits[j] = Q[i,e] * (K[i+j-S, e] + pb[j, e])   for |j-S| < S (strict)
        logits[j] = 0                                    for j in {0, 64} (K masked)
        logits[j] = -inf                                 for out-of-range positions
        ctx = sum_j softmax(logits)[j] * V[i+j-S, e]
    out = sigmoid(Q)^2 * ctx

Sharding: 8 cores = (batch b in 0..3) x (sequence half h in 0..1).
The h=1 half is REVERSED on the host so that every core sees an identical
problem: a sequence edge at local position 0 and valid data through the
right halo.  This keeps the SPMD graph uniform (no per-core masking).

Device layout: channels on partitions (2 halves of 128), sequence on the
free axis.  Window shifts are free AP offsets.  Per window offset d:
  DVE:  l_d = (K<<d + pb[d]) * Q           (fused scalar_tensor_tensor)
  ACT:  E_d = exp(l_d)
  DVE:  EV_d = E_d * V<<d
  PE:   N += I.T @ EV_d ; D += I.T @ E_d   (identity matmuls accumulate in PSUM)
Final: out = sigmoid(Q)^2 * N / D.

The hot path runs in bf16 (measured end-to-end error ~8e-3 vs the 2e-2
gate): halves DVE/ACT element cost and avoids the PE's fp32 HI/LO
double-pass.  K and V also exist as 1-element-shifted copies so reads at
odd window offsets stay 4-byte aligned (keeps the DVE 2x packed mode).

Raw Bass (manual semaphores): this walrus build rejects Tile's generated
sync (multi-wait instructions), so engine programs and cumulative
wait_ge thresholds are written out explicitly.
"""

import contextlib

import ml_dtypes
import numpy as np

import concourse.bass as bass
import concourse.mybir as mybir
from concourse import bass_utils

B, L, E, S = 4, 1024, 256, 32
O = 512          # output positions per core
HALO = 32        # halo on each side of the output range
NH = O + 2 * HALO  # 576: local K/V/q array length
P = 128
W = 2 * S + 1
F32 = mybir.dt.float32
BF16 = mybir.dt.bfloat16
NPBF = ml_dtypes.bfloat16

NLB = 4   # logit buffers
NEB = 6   # exp buffers
NVB = 4   # exp*V buffers
EV_DELAY = 2  # EV mult lags the logit STT by this many iterations

TRACE = False
LAST_RESULTS = None
_DEBUG_TAP = None
_CACHE = {}

# hot-loop iteration space
ITERS = [(eh, d) for eh in range(2) for d in range(-S + 1, S)]
NIT = len(ITERS)  # 126

# ---- static semaphore bookkeeping ----
# sem_pe counts: 24 proj matmuls, then per eh: 4 init + (N, D) per d
PE_PROJ = 24


def pe_after_init(eh):
    return PE_PROJ + 130 * eh + 4


def pe_after_N(idx):
    eh = ITERS[idx][0]
    k = idx - 63 * eh
    return pe_after_init(eh) + 2 * k + 1


def pe_after_D(idx):
    return pe_after_N(idx) + 1


PE_TOTAL = PE_PROJ + 260

# ACT projection ops per group (eh, t, chunk): q:1+1, k:1+1, v:2+1
PRJ_OPS = [1, 1, 1, 1, 2, 1] * 2
PRJ_CUM = np.cumsum(PRJ_OPS).tolist()          # after each group
PRJ_EH = [7, 14]                               # after each eh's projections
N_LOADS = 12


def _act_reciprocal(scalar, out, in_):
    """activation(Reciprocal) without bass's accuracy guard; ~2^-12 rel
    error is fine against this problem's 2e-2 gate and it moves the
    division off the DVE."""
    nc = scalar.bass
    return scalar.add_instruction(
        mybir.InstActivation(
            name=nc.get_next_instruction_name(),
            func=mybir.ActivationFunctionType.Reciprocal,
            ins=[
                scalar.lower_ap(in_),
                mybir.ImmediateValue(dtype=mybir.dt.float32, value=0.0),
                mybir.ImmediateValue(dtype=mybir.dt.float32, value=1.0),
                mybir.ImmediateValue(dtype=mybir.dt.float32, value=0.0),
            ],
            outs=[scalar.lower_ap(out)],
        )
    )


def _build_nc():
    nc = bass.Bass("TRN2")

    qT_d = nc.dram_tensor("qT", [E, NH], BF16, kind="ExternalInput")
    w_d = {t: nc.dram_tensor(f"w{t}T", [E, E], BF16, kind="ExternalInput")
           for t in "qkv"}
    b_d = nc.dram_tensor("bias", [E, 3], F32, kind="ExternalInput")
    pb_d = nc.dram_tensor("pbT", [E, W], BF16, kind="ExternalInput")
    out_d = nc.dram_tensor("out", [E, O], F32, kind="ExternalOutput")

    add = mybir.AluOpType.add
    mult = mybir.AluOpType.mult
    AF = mybir.ActivationFunctionType

    ctx = contextlib.ExitStack()
    with ctx:
        sb = lambda name, shape, dt=BF16: ctx.enter_context(
            nc.sbuf_tensor(name, shape, dt))[:, :]
        ps = lambda name, shape: ctx.enter_context(
            nc.psum_tensor(name, shape, F32))[:, :]
        sem = lambda name: ctx.enter_context(nc.semaphore(name))

        qT = [sb(f"qT{kh}", [P, NH]) for kh in range(2)]
        wT = {(t, kh): sb(f"w{t}{kh}", [P, E])
              for t in "qkv" for kh in range(2)}
        pb = [sb(f"pb{eh}", [P, W]) for eh in range(2)]
        bs = [sb(f"bs{eh}", [P, 3], F32) for eh in range(2)]
        QKV = {(t, eh): sb(f"{t}{eh}", [P, NH])
               for t in "qkv" for eh in range(2)}
        # 1-element-shifted copies for odd window offsets (alignment)
        K1 = [sb(f"k1_{eh}", [P, NH]) for eh in range(2)]
        V1 = [sb(f"v1_{eh}", [P, NH]) for eh in range(2)]
        ident = sb("ident", [P, P])
        ones = sb("ones", [P, NH])
        lb = [sb(f"lb{i}", [P, O]) for i in range(NLB)]
        eb = [sb(f"eb{i}", [P, O]) for i in range(NEB)]
        vb = [sb(f"vb{i}", [P, O]) for i in range(NVB)]
        sig = [sb(f"sig{eh}", [P, O], F32) for eh in range(2)]
        rec = sb("rec", [P, O], F32)
        nr = sb("nr", [P, O], F32)
        ob = [sb(f"ob{eh}", [P, O], F32) for eh in range(2)]
        tapb = sb("tapb", [P, O], F32)

        prj_ps = [ps(f"prj_ps{i}", [P, O]) for i in range(2)]
        D_ps = [ps(f"D_ps{eh}", [P, O]) for eh in range(2)]
        N_ps = [ps(f"N_ps{eh}", [P, O]) for eh in range(2)]

        s_load = sem("s_load")
        s_gp = sem("s_gp")
        s_prj = sem("s_prj")
        s_lg = sem("s_lg")
        s_ex = sem("s_ex")
        s_ev = sem("s_ev")
        s_pe = sem("s_pe")
        s_sig = sem("s_sig")
        s_epi = sem("s_epi")
        s_od = sem("s_od")

        def k_sh(eh, d):
            """K window-shifted AP, 4B-aligned: even offsets from K, odd
            from the 1-shifted copy."""
            o = HALO + d
            if o % 2 == 0:
                return QKV["k", eh][:, o:o + O]
            return K1[eh][:, o - 1:o - 1 + O]

        def v_sh(eh, d):
            o = HALO + d
            if o % 2 == 0:
                return QKV["v", eh][:, o:o + O]
            return V1[eh][:, o - 1:o - 1 + O]

        # projection groups: (eh, t, (n0, nn))
        groups = [(eh, t, c) for eh in range(2) for t in "qkv"
                  for c in ((0, 512), (512, NH - 512))]

        with nc.Block() as block:

            @block.sync
            def _(sync):
                for kh in range(2):
                    sync.dma_start(out=qT[kh], in_=qT_d[kh * P:(kh + 1) * P, :]
                                   ).then_inc(s_load, 16)
                for t in "qkv":
                    for kh in range(2):
                        sync.dma_start(out=wT[t, kh],
                                       in_=w_d[t][kh * P:(kh + 1) * P, :]
                                       ).then_inc(s_load, 16)
                for eh in range(2):
                    sync.dma_start(out=pb[eh], in_=pb_d[eh * P:(eh + 1) * P, :]
                                   ).then_inc(s_load, 16)
                    sync.dma_start(out=bs[eh], in_=b_d[eh * P:(eh + 1) * P, :]
                                   ).then_inc(s_load, 16)
                if _DEBUG_TAP is None:
                    for eh in range(2):
                        sync.wait_ge(s_epi, eh + 1)
                        sync.dma_start(out=out_d[eh * P:(eh + 1) * P, :],
                                       in_=ob[eh]).then_inc(s_od, 16)
                    sync.wait_ge(s_od, 32)
                else:
                    sync.wait_ge(s_epi, 2)
                    tap = {
                        "D0": lambda: tapb,
                        "N0": lambda: tapb,
                        "sig0": lambda: sig[0],
                        "out0": lambda: ob[0],
                    }[_DEBUG_TAP]()
                    tw = tap.shape[1]
                    sync.dma_start(out=out_d[0:P, 0:tw], in_=tap
                                   ).then_inc(s_od, 16)
                    sync.wait_ge(s_od, 16)

            @block.gpsimd
            def _(gpsimd):
                gpsimd.memset(ident, 0.0)
                gpsimd.affine_select(
                    out=ident, in_=ident,
                    compare_op=mybir.AluOpType.not_equal,
                    fill=1.0, base=0, pattern=[[-1, P]], channel_multiplier=1,
                ).then_inc(s_gp, 1)
                gpsimd.memset(ones, 1.0)
                gpsimd.memset(ones[:, 0:HALO], 0.0).then_inc(s_gp, 1)
                # shifted K/V copies (1-input copies run near line rate here)
                for eh in range(2):
                    gpsimd.wait_ge(s_prj, PRJ_EH[eh])
                    gpsimd.tensor_copy(K1[eh][:, 0:NH - 1],
                                       QKV["k", eh][:, 1:NH])
                    gpsimd.tensor_copy(V1[eh][:, 0:NH - 1],
                                       QKV["v", eh][:, 1:NH]
                                       ).then_inc(s_gp, 1)

            @block.tensor
            def _(tensor):
                tensor.wait_ge(s_load, N_LOADS * 16)
                tensor.wait_ge(s_gp, 2)
                # projections: ping-pong over two PSUM banks
                for g, (eh, t, (n0, nn)) in enumerate(groups):
                    bank = prj_ps[g % 2]
                    if g >= 2:
                        tensor.wait_ge(s_prj, PRJ_CUM[g - 2])
                    for kh in range(2):
                        tensor.matmul(
                            bank[:, :nn],
                            wT[t, kh][:, eh * P:(eh + 1) * P],
                            qT[kh][:, n0:n0 + nn],
                            start=(kh == 0), stop=(kh == 1),
                        ).then_inc(s_pe, 1)
                # window accumulation
                for idx, (eh, d) in enumerate(ITERS):
                    if d == -S + 1:
                        V = QKV["v", eh]
                        tensor.wait_ge(s_prj, PRJ_EH[eh])
                        tensor.matmul(D_ps[eh], ident, ones[:, HALO:HALO + O],
                                      start=True, stop=False).then_inc(s_pe, 1)
                        tensor.matmul(D_ps[eh], ident, ones[:, 0:O],
                                      start=False, stop=False).then_inc(s_pe, 1)
                        tensor.matmul(N_ps[eh], ident, V[:, 0:O],
                                      start=True, stop=False).then_inc(s_pe, 1)
                        tensor.matmul(N_ps[eh], ident,
                                      V[:, 2 * HALO:2 * HALO + O],
                                      start=False, stop=False).then_inc(s_pe, 1)
                    last = d == S - 1
                    tensor.wait_ge(s_ev, idx + 1)
                    tensor.matmul(N_ps[eh], ident, vb[idx % NVB],
                                  start=False, stop=last).then_inc(s_pe, 1)
                    vs = max(0, -d)
                    tensor.matmul(D_ps[eh][:, vs:], ident,
                                  eb[idx % NEB][:, vs:],
                                  start=False, stop=last).then_inc(s_pe, 1)

            @block.vector
            def _(vector):
                def emit_ev(j):
                    ehj, dj = ITERS[j]
                    vector.wait_ge(s_ex, j + 1)
                    if j >= NVB:
                        vector.wait_ge(s_pe, pe_after_N(j - NVB))
                    vector.tensor_mul(
                        vb[j % NVB], eb[j % NEB], v_sh(ehj, dj),
                    ).then_inc(s_ev, 1)

                for idx, (eh, d) in enumerate(ITERS):
                    if d == -S + 1:
                        vector.wait_ge(s_prj, PRJ_EH[eh])
                        vector.wait_ge(s_gp, 3 + eh)  # shifted copies ready
                    if idx >= NLB:
                        vector.wait_ge(s_ex, idx - NLB + 1)
                    vector.scalar_tensor_tensor(
                        out=lb[idx % NLB],
                        in0=k_sh(eh, d),
                        scalar=pb[eh][:, d + S:d + S + 1],
                        in1=QKV["q", eh][:, HALO:HALO + O],
                        op0=add, op1=mult,
                    ).then_inc(s_lg, 1)
                    if idx >= EV_DELAY:
                        emit_ev(idx - EV_DELAY)
                for j in range(NIT - EV_DELAY, NIT):
                    emit_ev(j)

                # epilogue
                for eh in range(2):
                    vector.wait_ge(s_pe, PE_TOTAL)
                    if eh == 0 and _DEBUG_TAP == "D0":
                        vector.tensor_copy(tapb, D_ps[0])
                    if eh == 0 and _DEBUG_TAP == "N0":
                        vector.tensor_copy(tapb, N_ps[0])
                    vector.wait_ge(s_sig, 2 + eh + 1)  # sigmoids + this rec
                    vector.tensor_mul(nr, N_ps[eh], rec)
                    # out = sigmoid(Q)^2 * nr
                    vector.tensor_mul(nr, sig[eh], nr)
                    vector.tensor_mul(ob[eh], sig[eh], nr
                                      ).then_inc(s_epi, 1)

            @block.scalar
            def _(scalar):
                # projections: add bias, move PSUM -> SBUF
                for g, (eh, t, (n0, nn)) in enumerate(groups):
                    ti = "qkv".index(t)
                    bank = prj_ps[g % 2]
                    scalar.wait_ge(s_pe, 2 * (g + 1))
                    T_sb = QKV[t, eh]
                    if t == "v" and n0 == 0:
                        scalar.activation(T_sb[:, 0:HALO], bank[:, 0:HALO],
                                          AF.Copy).then_inc(s_prj, 1)
                        scalar.activation(
                            T_sb[:, HALO:nn], bank[:, HALO:nn], AF.Identity,
                            bias=bs[eh][:, ti:ti + 1], scale=1.0,
                        ).then_inc(s_prj, 1)
                    else:
                        scalar.activation(
                            T_sb[:, n0:n0 + nn], bank[:, :nn], AF.Identity,
                            bias=bs[eh][:, ti:ti + 1], scale=1.0,
                        ).then_inc(s_prj, 1)
                for idx, (eh, d) in enumerate(ITERS):
                    scalar.wait_ge(s_lg, idx + 1)
                    if idx >= NEB:
                        scalar.wait_ge(s_pe, pe_after_D(idx - NEB))
                    scalar.activation(eb[idx % NEB], lb[idx % NLB], AF.Exp
                                      ).then_inc(s_ex, 1)
                # sigmoids + reciprocals (Sigmoid set is loaded once, at the
                # end, after all Exp ops)
                for eh in range(2):
                    scalar.activation(sig[eh], QKV["q", eh][:, HALO:HALO + O],
                                      AF.Sigmoid).then_inc(s_sig, 1)
                for eh in range(2):
                    scalar.wait_ge(s_pe, PE_TOTAL)
                    if eh == 1:
                        scalar.wait_ge(s_epi, 1)  # DVE done reading rec
                    _act_reciprocal(scalar, rec, D_ps[eh]).then_inc(s_sig, 1)

            # NOTE on rec: eh=0 uses rec written by the first reciprocal;
            # the second reciprocal overwrites it only after the DVE's
            # s_epi>=1 signals eh=0's nr is computed.

    return nc


def _shard_inputs(q, Wq, bq, Wk, bk, Wv, bv, pos_bias):
    """Build per-core input maps. Core c = 2*b + h."""
    wqT = np.ascontiguousarray(Wq.T).astype(NPBF)
    wkT = np.ascontiguousarray(Wk.T).astype(NPBF)
    wvT = np.ascontiguousarray(Wv.T).astype(NPBF)
    bias = np.stack([bq, bk, bv], axis=1).astype(np.float32)  # [E, 3]
    pbT_f = np.ascontiguousarray(pos_bias.T).astype(NPBF)        # [E, W]
    pbT_r = np.ascontiguousarray(pos_bias[::-1].T).astype(NPBF)  # reversed

    in_maps = []
    for c in range(8):
        b, h = divmod(c, 2)
        qh = np.zeros((NH, E), np.float32)
        if h == 0:
            qh[HALO:] = q[b, 0:O + HALO]          # positions -32..543, pad<0
        else:
            qh[HALO:] = q[b, L - (O + HALO):][::-1]  # reversed right half
        in_maps.append({
            "qT": np.ascontiguousarray(qh.T).astype(NPBF),
            "wqT": wqT, "wkT": wkT, "wvT": wvT,
            "bias": bias,
            "pbT": pbT_f if h == 0 else pbT_r,
        })
    return in_maps


def _unshard(results):
    out = np.empty((B, L, E), np.float32)
    for c in range(8):
        b, h = divmod(c, 2)
        o_core = np.asarray(results[c]["out"], np.float32).T  # [O, E]
        if h == 0:
            out[b, 0:O] = o_core
        else:
            out[b, L - O:] = o_core[::-1]
    return out


def kernel(q, Wq, bq, Wk, bk, Wv, bv, pos_bias):
    global LAST_RESULTS
    q = np.asarray(q, np.float32)
    if "nc" not in _CACHE:
        _CACHE["nc"] = _build_nc()
    nc = _CACHE["nc"]
    in_maps = _shard_inputs(q, np.asarray(Wq), np.asarray(bq), np.asarray(Wk),
                            np.asarray(bk), np.asarray(Wv), np.asarray(bv),
                            np.asarray(pos_bias))
    res = bass_utils.run_bass_kernel_spmd(
        nc, in_maps, core_ids=list(range(8)), trace=TRACE,
    )
    LAST_RESULTS = res
    return _unshard(res.results)


# revision 23
# speedup vs baseline: 1.2497x; 1.2497x over previous
"""AFT-Local sparse attention kernel for Trainium2, SPMD over 8 NeuronCores.

Problem (B=4, L=1024, E=256, S=32):
    Q = q @ Wq.T + bq ; K = q @ Wk.T + bk ; V = q @ Wv.T + bv
    For each (b, i, e):  per-channel softmax over the 65-wide window
        logits[j] = Q[i,e] * (K[i+j-S, e] + pb[j, e])   for |j-S| < S (strict)
        logits[j] = 0                                    for j in {0, 64} (K masked)
        logits[j] = -inf                                 for out-of-range positions
        ctx = sum_j softmax(logits)[j] * V[i+j-S, e]
    out = sigmoid(Q)^2 * ctx

Sharding: 8 cores = (batch b in 0..3) x (sequence half h in 0..1).
The h=1 half is REVERSED on the host so that every core sees an identical
problem: a sequence edge at local position 0 and valid data through the
right halo.  This keeps the SPMD graph uniform (no per-core masking).

Device layout: channels on partitions (2 halves of 128), sequence on the
free axis.  Window shifts are free AP offsets.  Per window offset d:
  DVE:  l_d = (K<<d + pb[d]) * Q           (fused scalar_tensor_tensor)
  ACT:  E_d = exp(l_d)
  DVE:  EV_d = E_d * V<<d
  PE:   N += I.T @ EV_d ; D += I.T @ E_d   (identity matmuls accumulate in PSUM)
Final: out = sigmoid(Q)^2 * N / D.

The hot path runs in bf16 (measured end-to-end error ~8e-3 vs the 2e-2
gate): halves DVE/ACT element cost and avoids the PE's fp32 HI/LO
double-pass.  K and V also exist as 1-element-shifted copies so reads at
odd window offsets stay 4-byte aligned (keeps the DVE 2x packed mode).

Raw Bass (manual semaphores): this walrus build rejects Tile's generated
sync (multi-wait instructions), so engine programs and cumulative
wait_ge thresholds are written out explicitly.
"""

import contextlib

import ml_dtypes
import numpy as np

import concourse.bass as bass
import concourse.mybir as mybir
from concourse import bass_utils

B, L, E, S = 4, 1024, 256, 32
O = 512          # output positions per core
HALO = 32        # halo on each side of the output range
NH = O + 2 * HALO  # 576: local K/V/q array length
P = 128
W = 2 * S + 1
F32 = mybir.dt.float32
BF16 = mybir.dt.bfloat16
NPBF = ml_dtypes.bfloat16

NLB = 4   # logit buffers
NEB = 6   # exp buffers
NVB = 4   # exp*V buffers
EV_DELAY = 2  # EV mult lags the logit STT by this many iterations

TRACE = False
LAST_RESULTS = None
_DEBUG_TAP = None
_CACHE = {}

# hot-loop iteration space
ITERS = [(eh, d) for eh in range(2) for d in range(-S + 1, S)]
NIT = len(ITERS)  # 126

# ---- static semaphore bookkeeping ----
# sem_pe counts: 24 proj matmuls, then per eh: 4 init + (N, D) per d
PE_PROJ = 24


def pe_after_init(eh):
    return PE_PROJ + 130 * eh + 4


def pe_after_N(idx):
    eh = ITERS[idx][0]
    k = idx - 63 * eh
    return pe_after_init(eh) + 2 * k + 1


def pe_after_D(idx):
    return pe_after_N(idx) + 1


PE_TOTAL = PE_PROJ + 260

# ACT projection ops per group (eh, t, chunk): q:1+1, k:1+1, v:2+1
PRJ_OPS = [1, 1, 1, 1, 2, 1] * 2
PRJ_CUM = np.cumsum(PRJ_OPS).tolist()          # after each group
PRJ_EH = [7, 14]                               # after each eh's projections
N_LOADS = 12


def _act_reciprocal(scalar, out, in_):
    """activation(Reciprocal) without bass's accuracy guard; ~2^-12 rel
    error is fine against this problem's 2e-2 gate and it moves the
    division off the DVE."""
    nc = scalar.bass
    return scalar.add_instruction(
        mybir.InstActivation(
            name=nc.get_next_instruction_name(),
            func=mybir.ActivationFunctionType.Reciprocal,
            ins=[
                scalar.lower_ap(in_),
                mybir.ImmediateValue(dtype=mybir.dt.float32, value=0.0),
                mybir.ImmediateValue(dtype=mybir.dt.float32, value=1.0),
                mybir.ImmediateValue(dtype=mybir.dt.float32, value=0.0),
            ],
            outs=[scalar.lower_ap(out)],
        )
    )


def _build_nc():
    nc = bass.Bass("TRN2")

    qT_d = nc.dram_tensor("qT", [E, NH], BF16, kind="ExternalInput")
    w_d = {t: nc.dram_tensor(f"w{t}T", [E, E], BF16, kind="ExternalInput")
           for t in "qkv"}
    b_d = nc.dram_tensor("bias", [E, 3], F32, kind="ExternalInput")
    pb_d = nc.dram_tensor("pbT", [E, W], F32, kind="ExternalInput")
    out_d = nc.dram_tensor("out", [E, O], F32, kind="ExternalOutput")

    add = mybir.AluOpType.add
    mult = mybir.AluOpType.mult
    AF = mybir.ActivationFunctionType

    ctx = contextlib.ExitStack()
    with ctx:
        sb = lambda name, shape, dt=BF16: ctx.enter_context(
            nc.sbuf_tensor(name, shape, dt))[:, :]
        ps = lambda name, shape: ctx.enter_context(
            nc.psum_tensor(name, shape, F32))[:, :]
        sem = lambda name: ctx.enter_context(nc.semaphore(name))

        qT = [sb(f"qT{kh}", [P, NH]) for kh in range(2)]
        wT = {(t, kh): sb(f"w{t}{kh}", [P, E])
              for t in "qkv" for kh in range(2)}
        pb = [sb(f"pb{eh}", [P, W], F32) for eh in range(2)]
        bs = [sb(f"bs{eh}", [P, 3], F32) for eh in range(2)]
        QKV = {(t, eh): sb(f"{t}{eh}", [P, NH])
               for t in "qkv" for eh in range(2)}
        # 1-element-shifted copies for odd window offsets (alignment)
        K1 = [sb(f"k1_{eh}", [P, NH]) for eh in range(2)]
        V1 = [sb(f"v1_{eh}", [P, NH]) for eh in range(2)]
        ident = sb("ident", [P, P])
        ones = sb("ones", [P, NH])
        lb = [sb(f"lb{i}", [P, O]) for i in range(NLB)]
        tb = [sb(f"tb{i}", [P, O]) for i in range(2)]
        eb = [sb(f"eb{i}", [P, O]) for i in range(NEB)]
        vb = [sb(f"vb{i}", [P, O]) for i in range(NVB)]
        sig = [sb(f"sig{eh}", [P, O], F32) for eh in range(2)]
        rec = sb("rec", [P, O], F32)
        nr = sb("nr", [P, O], F32)
        ob = [sb(f"ob{eh}", [P, O], F32) for eh in range(2)]
        tapb = sb("tapb", [P, O], F32)

        prj_ps = [ps(f"prj_ps{i}", [P, O]) for i in range(2)]
        D_ps = [ps(f"D_ps{eh}", [P, O]) for eh in range(2)]
        N_ps = [ps(f"N_ps{eh}", [P, O]) for eh in range(2)]

        s_load = sem("s_load")
        s_gp = sem("s_gp")
        s_prj = sem("s_prj")
        s_lg = sem("s_lg")
        s_ex = sem("s_ex")
        s_ev = sem("s_ev")
        s_pe = sem("s_pe")
        s_sig = sem("s_sig")
        s_epi = sem("s_epi")
        s_od = sem("s_od")

        def k_sh(eh, d):
            """K window-shifted AP, 4B-aligned: even offsets from K, odd
            from the 1-shifted copy."""
            o = HALO + d
            if o % 2 == 0:
                return QKV["k", eh][:, o:o + O]
            return K1[eh][:, o - 1:o - 1 + O]

        def v_sh(eh, d):
            o = HALO + d
            if o % 2 == 0:
                return QKV["v", eh][:, o:o + O]
            return V1[eh][:, o - 1:o - 1 + O]

        # projection groups: (eh, t, (n0, nn))
        groups = [(eh, t, c) for eh in range(2) for t in "qkv"
                  for c in ((0, 512), (512, NH - 512))]

        with nc.Block() as block:

            @block.sync
            def _(sync):
                for kh in range(2):
                    sync.dma_start(out=qT[kh], in_=qT_d[kh * P:(kh + 1) * P, :]
                                   ).then_inc(s_load, 16)
                for t in "qkv":
                    for kh in range(2):
                        sync.dma_start(out=wT[t, kh],
                                       in_=w_d[t][kh * P:(kh + 1) * P, :]
                                       ).then_inc(s_load, 16)
                for eh in range(2):
                    sync.dma_start(out=pb[eh], in_=pb_d[eh * P:(eh + 1) * P, :]
                                   ).then_inc(s_load, 16)
                    sync.dma_start(out=bs[eh], in_=b_d[eh * P:(eh + 1) * P, :]
                                   ).then_inc(s_load, 16)
                if _DEBUG_TAP is None:
                    for eh in range(2):
                        sync.wait_ge(s_epi, eh + 1)
                        sync.dma_start(out=out_d[eh * P:(eh + 1) * P, :],
                                       in_=ob[eh]).then_inc(s_od, 16)
                    sync.wait_ge(s_od, 32)
                else:
                    sync.wait_ge(s_epi, 2)
                    tap = {
                        "D0": lambda: tapb,
                        "N0": lambda: tapb,
                        "sig0": lambda: sig[0],
                        "out0": lambda: ob[0],
                    }[_DEBUG_TAP]()
                    tw = tap.shape[1]
                    sync.dma_start(out=out_d[0:P, 0:tw], in_=tap
                                   ).then_inc(s_od, 16)
                    sync.wait_ge(s_od, 16)

            @block.gpsimd
            def _(gpsimd):
                gpsimd.memset(ident, 0.0)
                gpsimd.affine_select(
                    out=ident, in_=ident,
                    compare_op=mybir.AluOpType.not_equal,
                    fill=1.0, base=0, pattern=[[-1, P]], channel_multiplier=1,
                ).then_inc(s_gp, 1)
                gpsimd.memset(ones, 1.0)
                gpsimd.memset(ones[:, 0:HALO], 0.0).then_inc(s_gp, 1)
                # shifted K/V copies (1-input copies run near line rate here)
                for eh in range(2):
                    gpsimd.wait_ge(s_prj, PRJ_EH[eh])
                    gpsimd.tensor_copy(K1[eh][:, 0:NH - 1],
                                       QKV["k", eh][:, 1:NH])
                    gpsimd.tensor_copy(V1[eh][:, 0:NH - 1],
                                       QKV["v", eh][:, 1:NH]
                                       ).then_inc(s_gp, 1)

            @block.tensor
            def _(tensor):
                tensor.wait_ge(s_load, N_LOADS * 16)
                tensor.wait_ge(s_gp, 2)
                # projections: ping-pong over two PSUM banks
                for g, (eh, t, (n0, nn)) in enumerate(groups):
                    bank = prj_ps[g % 2]
                    if g >= 2:
                        tensor.wait_ge(s_prj, PRJ_CUM[g - 2])
                    for kh in range(2):
                        tensor.matmul(
                            bank[:, :nn],
                            wT[t, kh][:, eh * P:(eh + 1) * P],
                            qT[kh][:, n0:n0 + nn],
                            start=(kh == 0), stop=(kh == 1),
                        ).then_inc(s_pe, 1)
                # window accumulation
                for idx, (eh, d) in enumerate(ITERS):
                    if d == -S + 1:
                        V = QKV["v", eh]
                        tensor.wait_ge(s_prj, PRJ_EH[eh])
                        tensor.matmul(D_ps[eh], ident, ones[:, HALO:HALO + O],
                                      start=True, stop=False).then_inc(s_pe, 1)
                        tensor.matmul(D_ps[eh], ident, ones[:, 0:O],
                                      start=False, stop=False).then_inc(s_pe, 1)
                        tensor.matmul(N_ps[eh], ident, V[:, 0:O],
                                      start=True, stop=False).then_inc(s_pe, 1)
                        tensor.matmul(N_ps[eh], ident,
                                      V[:, 2 * HALO:2 * HALO + O],
                                      start=False, stop=False).then_inc(s_pe, 1)
                    last = d == S - 1
                    tensor.wait_ge(s_ev, idx + 1)
                    tensor.matmul(N_ps[eh], ident, vb[idx % NVB],
                                  start=False, stop=last).then_inc(s_pe, 1)
                    vs = max(0, -d)
                    tensor.matmul(D_ps[eh][:, vs:], ident,
                                  eb[idx % NEB][:, vs:],
                                  start=False, stop=last).then_inc(s_pe, 1)

            @block.vector
            def _(vector):
                def emit_ev(j):
                    ehj, dj = ITERS[j]
                    vector.wait_ge(s_ex, j + 1)
                    if j >= NVB:
                        vector.wait_ge(s_pe, pe_after_N(j - NVB))
                    vector.tensor_mul(
                        vb[j % NVB], eb[j % NEB], v_sh(ehj, dj),
                    ).then_inc(s_ev, 1)

                for idx, (eh, d) in enumerate(ITERS):
                    if d == -S + 1:
                        vector.wait_ge(s_prj, PRJ_EH[eh])
                        vector.wait_ge(s_gp, 3 + eh)  # shifted copies ready
                    if idx >= NLB:
                        vector.wait_ge(s_ex, idx - NLB + 1)
                    t_ = tb[idx % 2]
                    vector.tensor_scalar_add(t_, k_sh(eh, d),
                                             pb[eh][:, d + S:d + S + 1])
                    vector.tensor_mul(
                        lb[idx % NLB], t_,
                        QKV["q", eh][:, HALO:HALO + O],
                    ).then_inc(s_lg, 1)
                    if idx >= EV_DELAY:
                        emit_ev(idx - EV_DELAY)
                for j in range(NIT - EV_DELAY, NIT):
                    emit_ev(j)

                # epilogue
                for eh in range(2):
                    vector.wait_ge(s_pe, PE_TOTAL)
                    if eh == 0 and _DEBUG_TAP == "D0":
                        vector.tensor_copy(tapb, D_ps[0])
                    if eh == 0 and _DEBUG_TAP == "N0":
                        vector.tensor_copy(tapb, N_ps[0])
                    vector.wait_ge(s_sig, 2 + eh + 1)  # sigmoids + this rec
                    vector.tensor_mul(nr, N_ps[eh], rec)
                    # out = sigmoid(Q)^2 * nr
                    vector.tensor_mul(nr, sig[eh], nr)
                    vector.tensor_mul(ob[eh], sig[eh], nr
                                      ).then_inc(s_epi, 1)

            @block.scalar
            def _(scalar):
                # projections: add bias, move PSUM -> SBUF
                for g, (eh, t, (n0, nn)) in enumerate(groups):
                    ti = "qkv".index(t)
                    bank = prj_ps[g % 2]
                    scalar.wait_ge(s_pe, 2 * (g + 1))
                    T_sb = QKV[t, eh]
                    if t == "v" and n0 == 0:
                        scalar.activation(T_sb[:, 0:HALO], bank[:, 0:HALO],
                                          AF.Copy).then_inc(s_prj, 1)
                        scalar.activation(
                            T_sb[:, HALO:nn], bank[:, HALO:nn], AF.Identity,
                            bias=bs[eh][:, ti:ti + 1], scale=1.0,
                        ).then_inc(s_prj, 1)
                    else:
                        scalar.activation(
                            T_sb[:, n0:n0 + nn], bank[:, :nn], AF.Identity,
                            bias=bs[eh][:, ti:ti + 1], scale=1.0,
                        ).then_inc(s_prj, 1)
                for idx, (eh, d) in enumerate(ITERS):
                    scalar.wait_ge(s_lg, idx + 1)
                    if idx >= NEB:
                        scalar.wait_ge(s_pe, pe_after_D(idx - NEB))
                    scalar.activation(eb[idx % NEB], lb[idx % NLB], AF.Exp
                                      ).then_inc(s_ex, 1)
                # sigmoids + reciprocals (Sigmoid set is loaded once, at the
                # end, after all Exp ops)
                for eh in range(2):
                    scalar.activation(sig[eh], QKV["q", eh][:, HALO:HALO + O],
                                      AF.Sigmoid).then_inc(s_sig, 1)
                for eh in range(2):
                    scalar.wait_ge(s_pe, PE_TOTAL)
                    if eh == 1:
                        scalar.wait_ge(s_epi, 1)  # DVE done reading rec
                    _act_reciprocal(scalar, rec, D_ps[eh]).then_inc(s_sig, 1)

            # NOTE on rec: eh=0 uses rec written by the first reciprocal;
            # the second reciprocal overwrites it only after the DVE's
            # s_epi>=1 signals eh=0's nr is computed.

    return nc


def _shard_inputs(q, Wq, bq, Wk, bk, Wv, bv, pos_bias):
    """Build per-core input maps. Core c = 2*b + h."""
    wqT = np.ascontiguousarray(Wq.T).astype(NPBF)
    wkT = np.ascontiguousarray(Wk.T).astype(NPBF)
    wvT = np.ascontiguousarray(Wv.T).astype(NPBF)
    bias = np.stack([bq, bk, bv], axis=1).astype(np.float32)  # [E, 3]
    pbT_f = np.ascontiguousarray(pos_bias.T).astype(np.float32)  # [E, W]
    pbT_r = np.ascontiguousarray(pos_bias[::-1].T).astype(np.float32)  # reversed

    in_maps = []
    for c in range(8):
        b, h = divmod(c, 2)
        qh = np.zeros((NH, E), np.float32)
        if h == 0:
            qh[HALO:] = q[b, 0:O + HALO]          # positions -32..543, pad<0
        else:
            qh[HALO:] = q[b, L - (O + HALO):][::-1]  # reversed right half
        in_maps.append({
            "qT": np.ascontiguousarray(qh.T).astype(NPBF),
            "wqT": wqT, "wkT": wkT, "wvT": wvT,
            "bias": bias,
            "pbT": pbT_f if h == 0 else pbT_r,
        })
    return in_maps


def _unshard(results):
    out = np.empty((B, L, E), np.float32)
    for c in range(8):
        b, h = divmod(c, 2)
        o_core = np.asarray(results[c]["out"], np.float32).T  # [O, E]
        if h == 0:
            out[b, 0:O] = o_core
        else:
            out[b, L - O:] = o_core[::-1]
    return out


def kernel(q, Wq, bq, Wk, bk, Wv, bv, pos_bias):
    global LAST_RESULTS
    q = np.asarray(q, np.float32)
    if "nc" not in _CACHE:
        _CACHE["nc"] = _build_nc()
    nc = _CACHE["nc"]
    in_maps = _shard_inputs(q, np.asarray(Wq), np.asarray(bq), np.asarray(Wk),
                            np.asarray(bk), np.asarray(Wv), np.asarray(bv),
                            np.asarray(pos_bias))
    res = bass_utils.run_bass_kernel_spmd(
        nc, in_maps, core_ids=list(range(8)), trace=TRACE,
    )
    LAST_RESULTS = res
    return _unshard(res.results)
